# revision 5
# baseline (speedup 1.0000x reference)
"""DisentangledSelfAttention (DeBERTa-style) Trainium2 Bass kernel.

Sharding: data-parallel over batch B=8 -> one batch element per NeuronCore.
Positional tensors are batch-independent and computed (replicated) per core.

Key algebraic structure exploited:
  rel[i, j] = j - i + 511 depends only on (j - i), and for S=384 only
  rel indices 128..894 (767 values) are ever used.  So:
    Kp_flat[p]  = rel_pos_emb[128 + p] @ Wpk          (p in [0, 767))
    Qp_flat[p]  = rel_pos_emb[128 + p] @ Wpq
    c2p[b,h,i,j] = q[b,i,h] . Kp_flat[j-i+383, h]
    p2c[b,h,i,j] = Qp_flat[j-i+383, h] . k[b,j,h]
  c2p comes from qp[i,p] = q[i].Kp_flat[p] by reading the flat [384,767]
  array with row pitch 766 instead of 767 (a strided "skew" read), and
  p2c likewise from kq[j,p'] = k[j].Qp_rev[p'] (Qp rows reversed) read
  transposed [j, i] with the same pitch-766 trick, then PE-transposed and
  accumulated straight into the score PSUM tile.

The softmax scale (dh**-0.5) and q_bias/v_bias are folded into the weights
and biases on the host before upload.
"""

import os
import sys

import numpy as np

B, S, D, H = 8, 384, 768, 12
DH = D // H          # 64
MAX_POS = 512
NP = 767             # number of used relative positions (128..894)
SCALE = DH ** -0.5

NIT = S // 128       # 3 i/j tiles
NDT = D // 128       # 6 d tiles
# chunks of the 767-wide positional axis (PSUM free dim <= 512 fp32)
PCHUNKS = [(0, 384), (384, 383)]

_CACHE = {}


def _import_concourse():
    try:
        import concourse.bass  # noqa: F401
    except ImportError:
        for p in ("/opt/trn_rl_repo", "/root/.axon_site/_ro/trn_rl_repo"):
            if os.path.isdir(p) and p not in sys.path:
                sys.path.insert(0, p)
        import concourse.bass  # noqa: F401


def _build(mm_dtype_name: str):
    """Build + finalize the per-core Bass program (identical on all cores)."""
    _import_concourse()
    import concourse.bass as bass
    import concourse.bacc as bacc
    import concourse.mybir as mybir
    import concourse.tile as tile
    from concourse.bass import ts
    from concourse.masks import make_identity
    from concourse.tile import add_dep_helper

    f32 = mybir.dt.float32
    mm_dt = getattr(mybir.dt, mm_dtype_name)
    ADD = mybir.AluOpType.add
    EXP = mybir.ActivationFunctionType.Exp

    def mm(ap):
        # reinterpret matmul operands (same 4-byte elements) for perf mode
        return ap.bitcast(mm_dt) if mm_dt != f32 else ap

    nc = bacc.Bacc("TRN2", target_bir_lowering=False, debug=False)

    # ---------------- DRAM I/O ----------------
    xT = nc.dram_tensor("xT", [D, S], f32, kind="ExternalInput")
    wq = nc.dram_tensor("wq", [D, D], f32, kind="ExternalInput")
    wk = nc.dram_tensor("wk", [D, D], f32, kind="ExternalInput")
    wv = nc.dram_tensor("wv", [D, D], f32, kind="ExternalInput")
    wpk = nc.dram_tensor("wpk", [D, D], f32, kind="ExternalInput")
    wpq = nc.dram_tensor("wpq", [D, D], f32, kind="ExternalInput")
    wo = nc.dram_tensor("wo", [D, D], f32, kind="ExternalInput")
    bq = nc.dram_tensor("bq", [D], f32, kind="ExternalInput")
    bk = nc.dram_tensor("bk", [D], f32, kind="ExternalInput")
    bv = nc.dram_tensor("bv", [D], f32, kind="ExternalInput")
    bo = nc.dram_tensor("bo", [D], f32, kind="ExternalInput")
    relkT = nc.dram_tensor("relkT", [D, NP], f32, kind="ExternalInput")
    relqTr = nc.dram_tensor("relqTr", [D, NP], f32, kind="ExternalInput")
    out = nc.dram_tensor("out", [S, D], f32, kind="ExternalOutput")

    # per-head DRAM scratch for the skew bounce
    qp_dram = [nc.dram_tensor(f"qp_scratch_{h}", [S, NP], f32) for h in range(H)]
    kq_dram = [nc.dram_tensor(f"kq_scratch_{h}", [S, NP], f32) for h in range(H)]

    with tile.TileContext(nc) as tc:
        with (
            tc.tile_pool(name="const", bufs=1) as constp,
            tc.tile_pool(name="big", bufs=1) as bigp,
            tc.tile_pool(name="wpool", bufs=2) as wpool,
            tc.tile_pool(name="psA", bufs=3, space="PSUM") as psA,
            tc.tile_pool(name="psWT", bufs=2, space="PSUM") as psWT,
            tc.tile_pool(name="psAV", bufs=2, space="PSUM") as psAV,
        ):
            def psum(tag, shape=None):
                pool = {"ps": psA, "wtps": psWT, "avps": psAV}[tag]
                return pool.tile(shape or [128, 384], f32, tag=tag, name=tag)

            ident = constp.tile([128, 128], f32, tag="ident")
            make_identity(nc, ident[:])

            bq_sb = constp.tile([128, NDT], f32, tag="bq")
            bk_sb = constp.tile([128, NDT], f32, tag="bk")
            bvf = constp.tile([1, D], f32, tag="bv")
            bof = constp.tile([1, D], f32, tag="bo")
            bvr = constp.tile([128, D], f32, tag="bvr")
            bor = constp.tile([128, D], f32, tag="bor")
            nc.sync.dma_start(bq_sb[:], bq[:].rearrange("(o p) -> p o", p=128))
            nc.sync.dma_start(bk_sb[:], bk[:].rearrange("(o p) -> p o", p=128))
            nc.sync.dma_start(bvf[:], bv[:].unsqueeze(0))
            nc.sync.dma_start(bof[:], bo[:].unsqueeze(0))
            nc.gpsimd.partition_broadcast(bvr[:], bvf[:])
            nc.gpsimd.partition_broadcast(bor[:], bof[:])

            qT_sb = bigp.tile([128, NDT, S], f32, tag="qT")
            kT_sb = bigp.tile([128, NDT, S], f32, tag="kT")
            v_sb = bigp.tile([128, NIT, D], f32, tag="v")
            KpT_sb = bigp.tile([128, NDT, NP], f32, tag="KpT")
            QpTr_sb = bigp.tile([128, NDT, NP], f32, tag="QpTr")
            attnT_sb = bigp.tile([128, NDT, S], f32, tag="attnT")

            # ---------- stage 1+2: projections & positional projections ----
            with tc.tile_pool(name="bigtmp", bufs=2) as bigtmp:
                xT_sb = bigtmp.tile([128, NDT, S], f32, tag="bigtmp")
                nc.sync.dma_start(
                    xT_sb[:], xT[:].rearrange("(o p) i -> p o i", p=128)
                )

                # q^T and k^T : [dout(part), i]  (bias per-partition)
                for wdram, bias_sb, dst in ((wq, bq_sb, qT_sb), (wk, bk_sb, kT_sb)):
                    w_sb = wpool.tile([128, NDT, D], f32, tag="w")
                    nc.sync.dma_start(
                        w_sb[:], wdram[:].rearrange("(o p) n -> p o n", p=128)
                    )
                    for mo in range(NDT):
                        ps_t = psum("ps")
                        for ko in range(NDT):
                            nc.tensor.matmul(
                                ps_t[:, :S],
                                mm(w_sb[:, ko, ts(mo, 128)]),
                                mm(xT_sb[:, ko, :]),
                                start=(ko == 0),
                                stop=(ko == NDT - 1),
                            )
                        nc.vector.tensor_scalar_add(
                            dst[:, mo, :], ps_t[:, :S], bias_sb[:, mo : mo + 1]
                        )

                # v : [i(part), dout]  (bias along free dim)
                w_sb = wpool.tile([128, NDT, D], f32, tag="w")
                nc.sync.dma_start(
                    w_sb[:], wv[:].rearrange("(o p) n -> p o n", p=128)
                )
                for io in range(NIT):
                    for no in range(2):
                        ps_t = psum("ps")
                        for ko in range(NDT):
                            nc.tensor.matmul(
                                ps_t[:, :384],
                                mm(xT_sb[:, ko, ts(io, 128)]),
                                mm(w_sb[:, ko, ts(no, 384)]),
                                start=(ko == 0),
                                stop=(ko == NDT - 1),
                            )
                        nc.vector.tensor_tensor(
                            v_sb[:, io, ts(no, 384)],
                            ps_t[:, :384],
                            bvr[:, ts(no, 384)],
                            ADD,
                        )

                # Kp^T and QpRev^T : [dout(part), p]
                for idx, (wdram, reldram, dst) in enumerate(
                    ((wpk, relkT, KpT_sb), (wpq, relqTr, QpTr_sb))
                ):
                    w_sb = wpool.tile([128, NDT, D], f32, tag="w")
                    nc.sync.dma_start(
                        w_sb[:], wdram[:].rearrange("(o p) n -> p o n", p=128)
                    )
                    rel_sb = bigtmp.tile([128, NDT, NP], f32, tag="bigtmp")
                    nc.sync.dma_start(
                        rel_sb[:], reldram[:].rearrange("(o p) n -> p o n", p=128)
                    )
                    for mo in range(NDT):
                        for ci, (cs, csz) in enumerate(PCHUNKS):
                            ps_t = psum("ps")
                            for ko in range(NDT):
                                nc.tensor.matmul(
                                    ps_t[:, :csz],
                                    mm(w_sb[:, ko, ts(mo, 128)]),
                                    mm(rel_sb[:, ko, cs : cs + csz]),
                                    start=(ko == 0),
                                    stop=(ko == NDT - 1),
                                )
                            if (mo + ci) % 2 == 0:
                                nc.vector.tensor_copy(
                                    dst[:, mo, cs : cs + csz], ps_t[:, :csz]
                                )
                            else:
                                nc.scalar.copy(
                                    dst[:, mo, cs : cs + csz], ps_t[:, :csz]
                                )

            # ---------- stages 3-5: attention per head ---------------------
            with (
                tc.tile_pool(name="work", bufs=2) as workp,
                tc.tile_pool(name="small", bufs=4) as smallp,
            ):
                qp_w = [[None] * NIT for _ in range(H)]
                kq_w = [[None] * NIT for _ in range(H)]

                def head_slices(h):
                    hp = 64 * (h % 2)
                    ho = h // 2
                    return hp, ho

                def stage3(pair):
                    """qp/kq matmuls + bounce to DRAM for heads 2p, 2p+1."""
                    for which in range(2):  # 0 -> qp, 1 -> kq
                        for it in range(NIT):
                            sb_tiles = {}
                            for sub in range(2):
                                h = 2 * pair + sub
                                hp, ho = head_slices(h)
                                lhsT = (qT_sb if which == 0 else kT_sb)[
                                    hp : hp + 64, ho, ts(it, 128)
                                ]
                                rhs_full = (KpT_sb if which == 0 else QpTr_sb)[
                                    hp : hp + 64, ho, :
                                ]
                                sb = workp.tile(
                                    [128, NP], f32, tag=f"bounce{which}"
                                )
                                sb_tiles[sub] = sb
                                for cs, csz in PCHUNKS:
                                    ps_t = psum("ps")
                                    nc.tensor.matmul(
                                        ps_t[:, :csz],
                                        mm(lhsT),
                                        mm(rhs_full[:, cs : cs + csz]),
                                        start=True,
                                        stop=True,
                                    )
                                    if sub == 0:
                                        nc.vector.tensor_copy(
                                            sb[:, cs : cs + csz], ps_t[:, :csz]
                                        )
                                    else:
                                        nc.scalar.copy(
                                            sb[:, cs : cs + csz], ps_t[:, :csz]
                                        )
                            for sub in range(2):
                                h = 2 * pair + sub
                                dram = (qp_dram if which == 0 else kq_dram)[h]
                                w_inst = nc.sync.dma_start(
                                    dram[ts(it, 128), :], sb_tiles[sub][:]
                                )
                                if which == 0:
                                    qp_w[h][it] = w_inst
                                else:
                                    kq_w[h][it] = w_inst

                def stage45(pair):
                    import concourse.bass as bass_mod

                    for sub in range(2):
                        h = 2 * pair + sub
                        hp, ho = head_slices(h)
                        wT_sb = workp.tile([128, NIT, S], f32, tag="wT")
                        for t in range(NIT):
                            # ---- scores psum: c2c + accumulated p2c transposes
                            sc_ps = psum("ps")
                            nc.tensor.matmul(
                                sc_ps[:, :S],
                                mm(qT_sb[hp : hp + 64, ho, ts(t, 128)]),
                                mm(kT_sb[hp : hp + 64, ho, :]),
                                start=True,
                                stop=False,
                                skip_group_check=True,
                            )
                            for u in range(NIT):
                                p2ct = smallp.tile([128, 128], f32, tag="p2ct")
                                r_inst = nc.sync.dma_start(
                                    p2ct[:],
                                    bass_mod.AP(
                                        kq_dram[h],
                                        383 + 766 * 128 * u + 128 * t,
                                        [[766, 128], [1, 128]],
                                    ),
                                )
                                add_dep_helper(
                                    r_inst.ins, kq_w[h][u].ins, reason="kq bounce"
                                )
                                nc.tensor.matmul(
                                    sc_ps[:, ts(u, 128)],
                                    p2ct[:],
                                    ident[:],
                                    is_transpose=True,
                                    start=False,
                                    stop=(u == NIT - 1),
                                    skip_group_check=True,
                                )
                            # ---- c2p diagonal + assemble + softmax
                            c2pt = workp.tile([128, S], f32, tag="c2p")
                            r_inst = nc.sync.dma_start(
                                c2pt[:],
                                bass_mod.AP(
                                    qp_dram[h],
                                    383 + 766 * 128 * t,
                                    [[766, 128], [1, 384]],
                                ),
                            )
                            add_dep_helper(
                                r_inst.ins, qp_w[h][t].ins, reason="qp bounce"
                            )
                            exp_sb = workp.tile([128, S], f32, tag="exp")
                            nc.vector.tensor_tensor(
                                exp_sb[:], sc_ps[:, :S], c2pt[:], ADD
                            )
                            ssum = smallp.tile([128, 1], f32, tag="ssum")
                            sinv = smallp.tile([128, 1], f32, tag="sinv")
                            nc.scalar.activation(
                                exp_sb[:], exp_sb[:], EXP, accum_out=ssum[:]
                            )
                            nc.vector.reciprocal(sinv[:], ssum[:])
                            nc.vector.tensor_scalar_mul(
                                exp_sb[:], exp_sb[:], sinv[:]
                            )
                            # ---- transpose normalized weights -> wT[j, i]
                            for u in range(NIT):
                                wt_ps = psum("wtps", shape=[128, 128])
                                nc.tensor.matmul(
                                    wt_ps[:],
                                    exp_sb[:, ts(u, 128)],
                                    ident[:],
                                    is_transpose=True,
                                )
                                if u % 2 == 0:
                                    nc.vector.tensor_copy(
                                        wT_sb[:, u, ts(t, 128)], wt_ps[:]
                                    )
                                else:
                                    nc.scalar.copy(
                                        wT_sb[:, u, ts(t, 128)], wt_ps[:]
                                    )
                        # ---- stage 5: AV for this head -> attnT
                        av_ps = psum("avps")
                        for u in range(NIT):
                            nc.tensor.matmul(
                                av_ps[hp : hp + 64, :S],
                                mm(v_sb[:, u, h * DH : (h + 1) * DH]),
                                mm(wT_sb[:, u, :]),
                                start=(u == 0),
                                stop=(u == NIT - 1),
                            )
                        nc.vector.tensor_copy(
                            attnT_sb[hp : hp + 64, ho, :], av_ps[hp : hp + 64, :S]
                        )

                # software pipeline: keep PE fed while head-pair bounces land
                stage3(0)
                for pair in range(6):
                    if pair + 1 < 6:
                        stage3(pair + 1)
                    stage45(pair)

                # ---------- stage 6: output projection --------------------
                w_sb = wpool.tile([128, NDT, D], f32, tag="w")
                nc.sync.dma_start(
                    w_sb[:], wo[:].rearrange("(o p) n -> p o n", p=128)
                )
                for io in range(NIT):
                    for no in range(2):
                        ps_t = psum("ps")
                        for ko in range(NDT):
                            nc.tensor.matmul(
                                ps_t[:, :384],
                                mm(attnT_sb[:, ko, ts(io, 128)]),
                                mm(w_sb[:, ko, ts(no, 384)]),
                                start=(ko == 0),
                                stop=(ko == NDT - 1),
                            )
                        o_sb = workp.tile([128, 384], f32, tag="osb")
                        nc.vector.tensor_tensor(
                            o_sb[:],
                            ps_t[:, :384],
                            bor[:, ts(no, 384)],
                            ADD,
                        )
                        nc.sync.dma_start(
                            out[ts(io, 128), ts(no, 384)], o_sb[:]
                        )

    nc.finalize()
    return nc


def _get_program(mm_dtype_name):
    key = ("nc", mm_dtype_name)
    if key not in _CACHE:
        _CACHE[key] = _build(mm_dtype_name)
    return _CACHE[key]


def _host_prep(inputs):
    f = np.float32
    x = np.asarray(inputs["x"], f)
    rel = np.asarray(inputs["rel_pos_emb"], f)
    rel_used = rel[MAX_POS - S : MAX_POS - S + NP]          # rows 128..894
    base = {
        "wq": np.ascontiguousarray(np.asarray(inputs["Wq"], f) * SCALE),
        "wk": np.ascontiguousarray(np.asarray(inputs["Wk"], f)),
        "wv": np.ascontiguousarray(np.asarray(inputs["Wv"], f)),
        "wpk": np.ascontiguousarray(np.asarray(inputs["Wpk"], f)),
        "wpq": np.ascontiguousarray(np.asarray(inputs["Wpq"], f) * SCALE),
        "wo": np.ascontiguousarray(np.asarray(inputs["Wo"], f)),
        "bq": ((np.asarray(inputs["bq"], f) + np.asarray(inputs["q_bias"], f))
               * SCALE).astype(f),
        "bk": np.asarray(inputs["bk"], f),
        "bv": (np.asarray(inputs["bv"], f) + np.asarray(inputs["v_bias"], f)
               ).astype(f),
        "bo": np.asarray(inputs["bo"], f),
        "relkT": np.ascontiguousarray(rel_used.T),
        "relqTr": np.ascontiguousarray(rel_used[::-1].T),
    }
    in_maps = []
    for b in range(B):
        m = dict(base)
        m["xT"] = np.ascontiguousarray(x[b].T)
        in_maps.append(m)
    return in_maps


def _run(inputs, trace=False):
    _import_concourse()
    from concourse.bass_utils import run_bass_kernel_spmd

    mm_dtype = os.environ.get("BASS_MM_DTYPE", "float32")
    nc = _get_program(mm_dtype)
    in_maps = _host_prep(inputs)
    res = run_bass_kernel_spmd(nc, in_maps, list(range(B)), trace=trace)
    outs = np.stack([np.asarray(res.results[b]["out"]) for b in range(B)])
    return outs.astype(np.float32), res


def kernel(**inputs) -> np.ndarray:
    out, _ = _run(inputs)
    return out


# revision 15
# speedup vs baseline: 2.5242x; 2.5242x over previous
"""DisentangledSelfAttention (DeBERTa-style) Trainium2 Bass kernel.

Sharding: data-parallel over batch B=8 -> one batch element per NeuronCore.
Positional tensors are batch-independent and computed (replicated) per core.

Key algebraic structure exploited:
  rel[i, j] = j - i + 511 depends only on (j - i), and for S=384 only
  rel indices 128..894 (767 values) are ever used.  So:
    Kp_flat[p]  = rel_pos_emb[128 + p] @ Wpk          (p in [0, 767))
    Qp_flat[p]  = rel_pos_emb[128 + p] @ Wpq
    c2p[b,h,i,j] = q[b,i,h] . Kp_flat[j-i+383, h]
    p2c[b,h,i,j] = Qp_flat[j-i+383, h] . k[b,j,h]
  c2p comes from qp[i,p] = q[i].Kp_flat[p]: each 128-row i-tile computes a
  512-wide window of qp, bounces it to DRAM [384x512], and reads the score
  block back with row pitch 511 instead of 512 (a strided "skew" read that
  turns the per-row diagonal shift into a flat 2D access pattern).  p2c
  likewise from kq[j,p'] = k[j].Qp_rev[p'] (Qp rows reversed), read back
  transposed [j, i] with the same skew trick, then PE-transposed and
  accumulated straight into the score PSUM tile (transposes are matmuls,
  so has_written accumulation is free).

The softmax scale (dh**-0.5) and q_bias/v_bias are folded into the weights
and biases on the host before upload.
"""

import os
import sys

import numpy as np

B, S, D, H = 8, 384, 768, 12
DH = D // H          # 64
MAX_POS = 512
NP = 767             # number of used relative positions (128..894)
SCALE = DH ** -0.5

NIT = S // 128       # 3 i/j tiles
NDT = D // 128       # 6 d tiles
# chunks of the 767-wide positional axis (PSUM free dim <= 512 fp32)
PCHUNKS = [(0, 384), (384, 383)]

_CACHE = {}


def _import_concourse():
    try:
        import concourse.bass  # noqa: F401
    except ImportError:
        for p in ("/opt/trn_rl_repo", "/root/.axon_site/_ro/trn_rl_repo"):
            if os.path.isdir(p) and p not in sys.path:
                sys.path.insert(0, p)
        import concourse.bass  # noqa: F401


def _build(mm_dtype_name: str):
    """Build + finalize the per-core Bass program (identical on all cores)."""
    _import_concourse()
    import concourse.bass as bass
    import concourse.bacc as bacc
    import concourse.mybir as mybir
    import concourse.tile as tile
    from concourse.bass import ts
    from concourse.masks import make_identity
    from concourse.tile import add_dep_helper

    f32 = mybir.dt.float32
    mm_dt = getattr(mybir.dt, mm_dtype_name)
    ADD = mybir.AluOpType.add
    EXP = mybir.ActivationFunctionType.Exp

    def mm(ap):
        # reinterpret matmul operands (same 4-byte elements) for perf mode
        return ap.bitcast(mm_dt) if mm_dt != f32 else ap

    nc = bacc.Bacc("TRN2", target_bir_lowering=False, debug=False)

    # ---------------- DRAM I/O ----------------
    xT = nc.dram_tensor("xT", [D, S], f32, kind="ExternalInput")
    wq = nc.dram_tensor("wq", [D, D], f32, kind="ExternalInput")
    wk = nc.dram_tensor("wk", [D, D], f32, kind="ExternalInput")
    wv = nc.dram_tensor("wv", [D, D], f32, kind="ExternalInput")
    wpk = nc.dram_tensor("wpk", [D, D], f32, kind="ExternalInput")
    wpq = nc.dram_tensor("wpq", [D, D], f32, kind="ExternalInput")
    wo = nc.dram_tensor("wo", [D, D], f32, kind="ExternalInput")
    bq = nc.dram_tensor("bq", [D], f32, kind="ExternalInput")
    bk = nc.dram_tensor("bk", [D], f32, kind="ExternalInput")
    bv = nc.dram_tensor("bv", [D], f32, kind="ExternalInput")
    bo = nc.dram_tensor("bo", [D], f32, kind="ExternalInput")
    relkT = nc.dram_tensor("relkT", [D, NP], f32, kind="ExternalInput")
    relqTr = nc.dram_tensor("relqTr", [D, NP], f32, kind="ExternalInput")
    out = nc.dram_tensor("out", [S, D], f32, kind="ExternalOutput")

    # per-head DRAM scratch for the skew bounce
    qp_dram = [nc.dram_tensor(f"qp_scratch_{h}", [S, NP], f32) for h in range(H)]
    kq_dram = [nc.dram_tensor(f"kq_scratch_{h}", [S, NP], f32) for h in range(H)]

    with tile.TileContext(nc) as tc:
        with (
            tc.tile_pool(name="const", bufs=1) as constp,
            tc.tile_pool(name="big", bufs=1) as bigp,
            tc.tile_pool(name="wpool", bufs=2) as wpool,
            tc.tile_pool(name="psA", bufs=3, space="PSUM") as psA,
            tc.tile_pool(name="psWT", bufs=2, space="PSUM") as psWT,
            tc.tile_pool(name="psAV", bufs=2, space="PSUM") as psAV,
        ):
            def psum(tag, shape=None):
                pool = {"ps": psA, "wtps": psWT, "avps": psAV}[tag]
                return pool.tile(shape or [128, 384], f32, tag=tag, name=tag)

            ident = constp.tile([128, 128], f32, tag="ident")
            make_identity(nc, ident[:])

            bq_sb = constp.tile([128, NDT], f32, tag="bq")
            bk_sb = constp.tile([128, NDT], f32, tag="bk")
            bvf = constp.tile([1, D], f32, tag="bv")
            bof = constp.tile([1, D], f32, tag="bo")
            bvr = constp.tile([128, D], f32, tag="bvr")
            bor = constp.tile([128, D], f32, tag="bor")
            nc.sync.dma_start(bq_sb[:], bq[:].rearrange("(o p) -> p o", p=128))
            nc.sync.dma_start(bk_sb[:], bk[:].rearrange("(o p) -> p o", p=128))
            nc.sync.dma_start(bvf[:], bv[:].unsqueeze(0))
            nc.sync.dma_start(bof[:], bo[:].unsqueeze(0))
            nc.gpsimd.partition_broadcast(bvr[:], bvf[:])
            nc.gpsimd.partition_broadcast(bor[:], bof[:])

            qT_sb = bigp.tile([128, NDT, S], f32, tag="qT")
            kT_sb = bigp.tile([128, NDT, S], f32, tag="kT")
            v_sb = bigp.tile([128, NIT, D], f32, tag="v")
            KpT_sb = bigp.tile([128, NDT, NP], f32, tag="KpT")
            QpTr_sb = bigp.tile([128, NDT, NP], f32, tag="QpTr")
            attnT_sb = bigp.tile([128, NDT, S], f32, tag="attnT")

            # ---------- stage 1+2: projections & positional projections ----
            with tc.tile_pool(name="bigtmp", bufs=2) as bigtmp:
                xT_sb = bigtmp.tile([128, NDT, S], f32, tag="bigtmp")
                nc.sync.dma_start(
                    xT_sb[:], xT[:].rearrange("(o p) i -> p o i", p=128)
                )

                # q^T and k^T : [dout(part), i]  (bias per-partition)
                for wdram, bias_sb, dst in ((wq, bq_sb, qT_sb), (wk, bk_sb, kT_sb)):
                    w_sb = wpool.tile([128, NDT, D], f32, tag="w")
                    nc.sync.dma_start(
                        w_sb[:], wdram[:].rearrange("(o p) n -> p o n", p=128)
                    )
                    for mo in range(NDT):
                        ps_t = psum("ps")
                        for ko in range(NDT):
                            nc.tensor.matmul(
                                ps_t[:, :S],
                                mm(w_sb[:, ko, ts(mo, 128)]),
                                mm(xT_sb[:, ko, :]),
                                start=(ko == 0),
                                stop=(ko == NDT - 1),
                            )
                        nc.vector.tensor_scalar_add(
                            dst[:, mo, :], ps_t[:, :S], bias_sb[:, mo : mo + 1]
                        )

                # v : [i(part), dout]  (bias along free dim)
                w_sb = wpool.tile([128, NDT, D], f32, tag="w")
                nc.sync.dma_start(
                    w_sb[:], wv[:].rearrange("(o p) n -> p o n", p=128)
                )
                for io in range(NIT):
                    for no in range(2):
                        ps_t = psum("ps")
                        for ko in range(NDT):
                            nc.tensor.matmul(
                                ps_t[:, :384],
                                mm(xT_sb[:, ko, ts(io, 128)]),
                                mm(w_sb[:, ko, ts(no, 384)]),
                                start=(ko == 0),
                                stop=(ko == NDT - 1),
                            )
                        nc.vector.tensor_tensor(
                            v_sb[:, io, ts(no, 384)],
                            ps_t[:, :384],
                            bvr[:, ts(no, 384)],
                            ADD,
                        )

                # Kp^T and QpRev^T : [dout(part), p]
                for idx, (wdram, reldram, dst) in enumerate(
                    ((wpk, relkT, KpT_sb), (wpq, relqTr, QpTr_sb))
                ):
                    w_sb = wpool.tile([128, NDT, D], f32, tag="w")
                    nc.sync.dma_start(
                        w_sb[:], wdram[:].rearrange("(o p) n -> p o n", p=128)
                    )
                    rel_sb = bigtmp.tile([128, NDT, NP], f32, tag="bigtmp")
                    nc.sync.dma_start(
                        rel_sb[:], reldram[:].rearrange("(o p) n -> p o n", p=128)
                    )
                    for mo in range(NDT):
                        for ci, (cs, csz) in enumerate(PCHUNKS):
                            ps_t = psum("ps")
                            for ko in range(NDT):
                                nc.tensor.matmul(
                                    ps_t[:, :csz],
                                    mm(w_sb[:, ko, ts(mo, 128)]),
                                    mm(rel_sb[:, ko, cs : cs + csz]),
                                    start=(ko == 0),
                                    stop=(ko == NDT - 1),
                                )
                            if (mo + ci) % 2 == 0:
                                nc.vector.tensor_copy(
                                    dst[:, mo, cs : cs + csz], ps_t[:, :csz]
                                )
                            else:
                                nc.scalar.copy(
                                    dst[:, mo, cs : cs + csz], ps_t[:, :csz]
                                )

            # ---------- stages 3-5: attention per head ---------------------
            with (
                tc.tile_pool(name="work", bufs=3) as workp,
                tc.tile_pool(name="small", bufs=4) as smallp,
            ):
                qp_w = [[None] * NIT for _ in range(H)]
                kq_w = [[None] * NIT for _ in range(H)]

                def head_slices(h):
                    hp = 64 * (h % 2)
                    ho = h // 2
                    return hp, ho

                def stage3(pair):
                    """qp/kq matmuls + bounce to DRAM for heads 2p, 2p+1."""
                    for which in range(2):  # 0 -> qp, 1 -> kq
                        for it in range(NIT):
                            sb_tiles = {}
                            for sub in range(2):
                                h = 2 * pair + sub
                                hp, ho = head_slices(h)
                                lhsT = (qT_sb if which == 0 else kT_sb)[
                                    hp : hp + 64, ho, ts(it, 128)
                                ]
                                rhs_full = (KpT_sb if which == 0 else QpTr_sb)[
                                    hp : hp + 64, ho, :
                                ]
                                sb = workp.tile(
                                    [128, NP], f32, tag=f"bounce{which}"
                                )
                                sb_tiles[sub] = sb
                                for cs, csz in PCHUNKS:
                                    ps_t = psum("ps")
                                    nc.tensor.matmul(
                                        ps_t[:, :csz],
                                        mm(lhsT),
                                        mm(rhs_full[:, cs : cs + csz]),
                                        start=True,
                                        stop=True,
                                    )
                                    if sub == 0:
                                        nc.vector.tensor_copy(
                                            sb[:, cs : cs + csz], ps_t[:, :csz]
                                        )
                                    else:
                                        nc.scalar.copy(
                                            sb[:, cs : cs + csz], ps_t[:, :csz]
                                        )
                            for sub in range(2):
                                h = 2 * pair + sub
                                dram = (qp_dram if which == 0 else kq_dram)[h]
                                w_inst = nc.sync.dma_start(
                                    dram[ts(it, 128), :], sb_tiles[sub][:]
                                )
                                if which == 0:
                                    qp_w[h][it] = w_inst
                                else:
                                    kq_w[h][it] = w_inst

                def stage45(pair):
                    import concourse.bass as bass_mod

                    for sub in range(2):
                        h = 2 * pair + sub
                        hp, ho = head_slices(h)
                        wT_sb = workp.tile([128, NIT, S], f32, tag="wT")
                        for t in range(NIT):
                            # ---- scores psum: c2c + accumulated p2c transposes
                            sc_ps = psum("ps")
                            nc.tensor.matmul(
                                sc_ps[:, :S],
                                mm(qT_sb[hp : hp + 64, ho, ts(t, 128)]),
                                mm(kT_sb[hp : hp + 64, ho, :]),
                                start=True,
                                stop=False,
                                skip_group_check=True,
                            )
                            for u in range(NIT):
                                p2ct = smallp.tile([128, 128], f32, tag="p2ct")
                                r_inst = nc.sync.dma_start(
                                    p2ct[:],
                                    bass_mod.AP(
                                        kq_dram[h],
                                        383 + 766 * 128 * u + 128 * t,
                                        [[766, 128], [1, 128]],
                                    ),
                                )
                                add_dep_helper(
                                    r_inst.ins, kq_w[h][u].ins, reason="kq bounce"
                                )
                                nc.tensor.matmul(
                                    sc_ps[:, ts(u, 128)],
                                    p2ct[:],
                                    ident[:],
                                    is_transpose=True,
                                    start=False,
                                    stop=(u == NIT - 1),
                                    skip_group_check=True,
                                )
                            # ---- c2p diagonal + assemble + softmax
                            c2pt = workp.tile([128, S], f32, tag="c2p")
                            r_inst = nc.sync.dma_start(
                                c2pt[:],
                                bass_mod.AP(
                                    qp_dram[h],
                                    383 + 766 * 128 * t,
                                    [[766, 128], [1, 384]],
                                ),
                            )
                            add_dep_helper(
                                r_inst.ins, qp_w[h][t].ins, reason="qp bounce"
                            )
                            exp_sb = workp.tile([128, S], f32, tag="exp")
                            nc.vector.tensor_tensor(
                                exp_sb[:], sc_ps[:, :S], c2pt[:], ADD
                            )
                            ssum = smallp.tile([128, 1], f32, tag="ssum")
                            sinv = smallp.tile([128, 1], f32, tag="sinv")
                            nc.scalar.activation(
                                exp_sb[:], exp_sb[:], EXP, accum_out=ssum[:]
                            )
                            nc.vector.reciprocal(sinv[:], ssum[:])
                            nc.vector.tensor_scalar_mul(
                                exp_sb[:], exp_sb[:], sinv[:]
                            )
                            # ---- transpose normalized weights -> wT[j, i]
                            for u in range(NIT):
                                wt_ps = psum("wtps", shape=[128, 128])
                                nc.tensor.matmul(
                                    wt_ps[:],
                                    exp_sb[:, ts(u, 128)],
                                    ident[:],
                                    is_transpose=True,
                                )
                                if u % 2 == 0:
                                    nc.vector.tensor_copy(
                                        wT_sb[:, u, ts(t, 128)], wt_ps[:]
                                    )
                                else:
                                    nc.scalar.copy(
                                        wT_sb[:, u, ts(t, 128)], wt_ps[:]
                                    )
                        # ---- stage 5: AV for this head -> attnT
                        av_ps = psum("avps")
                        for u in range(NIT):
                            nc.tensor.matmul(
                                av_ps[hp : hp + 64, :S],
                                mm(v_sb[:, u, h * DH : (h + 1) * DH]),
                                mm(wT_sb[:, u, :]),
                                start=(u == 0),
                                stop=(u == NIT - 1),
                            )
                        nc.vector.tensor_copy(
                            attnT_sb[hp : hp + 64, ho, :], av_ps[hp : hp + 64, :S]
                        )

                # software pipeline: keep PE fed while head-pair bounces land
                stage3(0)
                for pair in range(6):
                    if pair + 1 < 6:
                        stage3(pair + 1)
                    stage45(pair)

                # ---------- stage 6: output projection --------------------
                w_sb = wpool.tile([128, NDT, D], f32, tag="w")
                nc.sync.dma_start(
                    w_sb[:], wo[:].rearrange("(o p) n -> p o n", p=128)
                )
                for io in range(NIT):
                    for no in range(2):
                        ps_t = psum("ps")
                        for ko in range(NDT):
                            nc.tensor.matmul(
                                ps_t[:, :384],
                                mm(attnT_sb[:, ko, ts(io, 128)]),
                                mm(w_sb[:, ko, ts(no, 384)]),
                                start=(ko == 0),
                                stop=(ko == NDT - 1),
                            )
                        o_sb = workp.tile([128, 384], f32, tag="osb")
                        nc.vector.tensor_tensor(
                            o_sb[:],
                            ps_t[:, :384],
                            bor[:, ts(no, 384)],
                            ADD,
                        )
                        nc.sync.dma_start(
                            out[ts(io, 128), ts(no, 384)], o_sb[:]
                        )

    nc.finalize()
    return nc


def _get_program(mm_dtype_name):
    key = ("nc", mm_dtype_name, os.environ.get("BASS_F32R_SITES", "abcdef"))
    if key not in _CACHE:
        _CACHE[key] = _build(mm_dtype_name)
    return _CACHE[key]


def _host_prep(inputs):
    f = np.float32
    x = np.asarray(inputs["x"], f)
    rel = np.asarray(inputs["rel_pos_emb"], f)
    rel_used = rel[MAX_POS - S : MAX_POS - S + NP]          # rows 128..894
    base = {
        "wq": np.ascontiguousarray(np.asarray(inputs["Wq"], f) * SCALE),
        "wk": np.ascontiguousarray(np.asarray(inputs["Wk"], f)),
        "wv": np.ascontiguousarray(np.asarray(inputs["Wv"], f)),
        "wpk": np.ascontiguousarray(np.asarray(inputs["Wpk"], f)),
        "wpq": np.ascontiguousarray(np.asarray(inputs["Wpq"], f) * SCALE),
        "wo": np.ascontiguousarray(np.asarray(inputs["Wo"], f)),
        "bq": ((np.asarray(inputs["bq"], f) + np.asarray(inputs["q_bias"], f))
               * SCALE).astype(f),
        "bk": np.asarray(inputs["bk"], f),
        "bv": (np.asarray(inputs["bv"], f) + np.asarray(inputs["v_bias"], f)
               ).astype(f),
        "bo": np.asarray(inputs["bo"], f),
        "relkT": np.ascontiguousarray(rel_used.T),
        "relqTr": np.ascontiguousarray(rel_used[::-1].T),
    }
    in_maps = []
    for b in range(B):
        m = dict(base)
        m["xT"] = np.ascontiguousarray(x[b].T)
        in_maps.append(m)
    return in_maps


def _get_runner():
    """Build (once) a jitted SPMD executor for the compiled program.

    Mirrors concourse.bass2jax.run_bass_via_pjrt's multi-core path but caches
    the jitted callable so repeated kernel() calls don't re-trace/re-compile.
    """
    key = "runner"
    if key in _CACHE:
        return _CACHE[key]
    _import_concourse()
    import jax
    import jax.numpy as jnp  # noqa: F401
    from jax.sharding import Mesh, PartitionSpec
    from jax.experimental.shard_map import shard_map
    import concourse.mybir as mybir
    from concourse import bass2jax

    mm_dtype = os.environ.get("BASS_MM_DTYPE", "float32r")
    nc = _get_program(mm_dtype)
    bass2jax.install_neuronx_cc_hook()

    partition_name = (
        nc.partition_id_tensor.name if nc.partition_id_tensor else None
    )
    in_names, out_names, out_avals, zero_outs = [], [], [], []
    for alloc in nc.m.functions[0].allocations:
        if not isinstance(alloc, mybir.MemoryLocationSet):
            continue
        name = alloc.memorylocations[0].name
        if alloc.kind == "ExternalInput":
            if name != partition_name:
                in_names.append(name)
        elif alloc.kind == "ExternalOutput":
            out_names.append(name)
            shape = tuple(alloc.tensor_shape)
            dtype = mybir.dt.np(alloc.dtype)
            out_avals.append(jax.core.ShapedArray(shape, dtype))
            zero_outs.append(np.zeros(shape, dtype))
    n_params = len(in_names)
    all_names = in_names + out_names
    if partition_name is not None:
        all_names = all_names + [partition_name]

    def _body(*args):
        operands = list(args)
        if partition_name is not None:
            operands.append(bass2jax.partition_id_tensor())
        outs = bass2jax._bass_exec_p.bind(
            *operands,
            out_avals=tuple(out_avals),
            in_names=tuple(all_names),
            out_names=tuple(out_names),
            lowering_input_output_aliases=(),
            sim_require_finite=True,
            sim_require_nnan=True,
            nc=nc,
        )
        return tuple(outs)

    devices = jax.devices()[:B]
    mesh = Mesh(np.asarray(devices), ("core",))
    n_outs = len(out_names)
    sharded = jax.jit(
        shard_map(
            _body,
            mesh=mesh,
            in_specs=(PartitionSpec("core"),) * (n_params + n_outs),
            out_specs=(PartitionSpec("core"),) * n_outs,
            check_rep=False,
        ),
        donate_argnums=tuple(range(n_params, n_params + n_outs)),
        keep_unused=True,
    )

    def run(in_maps):
        concat_in = [
            np.concatenate([np.asarray(in_maps[c][nm]) for c in range(B)], axis=0)
            for nm in in_names
        ]
        concat_zeros = [
            np.zeros((B * z.shape[0], *z.shape[1:]), z.dtype) for z in zero_outs
        ]
        out_arrs = sharded(*concat_in, *concat_zeros)
        return [
            {
                nm: np.asarray(out_arrs[i]).reshape(B, *out_avals[i].shape)[c]
                for i, nm in enumerate(out_names)
            }
            for c in range(B)
        ]

    _CACHE[key] = run
    return run


def _run(inputs, trace=False):
    run = _get_runner()
    in_maps = _host_prep(inputs)
    results = run(in_maps)
    outs = np.stack([np.asarray(results[b]["out"]) for b in range(B)])
    return outs.astype(np.float32), None


def kernel(**inputs) -> np.ndarray:
    out, _ = _run(inputs)
    return out
